# revision 2
# baseline (speedup 1.0000x reference)
"""GPT-3 style multi-head attention on Trainium2, 8-core SPMD Bass kernel.

Problem shapes: B=2, S=4096, D=768, H=12, depth=64 (fp32).

Sharding (hardcoded): core c in 0..7 -> batch b = c//4, head group g = c%4
(3 heads per core).  Per core, v2 schedule (ACT/exp is the bottleneck
engine, everything else hides under it):
  1. project V then K for all seq (PE transpose in f32r + f32r matmuls,
     DVE evacuations with fused bias),
  2. per 512-wide q-block: project Q for that block, then attention:
     QK^T for heads 0&1 issued as a row-tiled pair (K=64 each, PE array
     halves run concurrently), exp on ScalarE over both heads in one
     instruction, AV with an appended ones column for the softmax
     denominator; head 2 is self-paired over (even, odd) key chunks
     using partition-duplicated qT/kT built for free via duplicated
     weight columns,
  3. output projection emitted per q-block, interleaved under the next
     block's exp shadow; partial [4096, 768] -> DRAM.
Host sums the 4 partials per batch and adds the output bias bo.
"""

import numpy as np

import concourse.bacc as bacc
import concourse.mybir as mybir
import concourse.tile as tile
from concourse import bass_utils
from concourse.masks import make_identity

B, S, D, H = 2, 4096, 768, 12
DEPTH = 64
HPC = 3                 # heads per core
GW = HPC * DEPTH        # 192: head-group width
N_CORES = 8
SCALE = 1.0 / float(np.sqrt(DEPTH))

F32 = mybir.dt.float32
F32R = mybir.dt.float32r
AF = mybir.ActivationFunctionType

P = 128
FCH = D // P            # 6 feature chunks
NSP = S // (2 * P)      # 16 seq pairs (256 rows each)
NKC = S // P            # 32 key chunks
QB = 512                # q block width
NQB = S // QB           # 8

# set by test.py to get a traced run
TRACE = False
LAST_RESULTS = None


def _emit(nc, tc, ctx, tensors, repeat=1, phases="ABC"):
    setup = _emit_setup(nc, tc, ctx, tensors)
    for _ in range(repeat):
        _emit_compute(nc, tc, tensors, setup, phases=phases)


def _emit_setup(nc, tc, ctx, tensors):
    XQ, XK, XV, WQ, WK, WV, WO, BQ, BK, BV, OUT = tensors

    const = ctx.enter_context(tc.tile_pool(name="const", bufs=1))

    ident = const.tile([P, P], F32R)
    make_identity(nc, ident[:])

    # biases as per-partition columns; head-2 slices duplicated into both
    # partition halves
    bq01 = const.tile([P, 1], F32)
    nc.sync.dma_start(bq01[:], BQ[0:P, :])
    bq2d = const.tile([P, 1], F32)
    nc.sync.dma_start(bq2d[0:DEPTH, :], BQ[P:GW, :])
    nc.sync.dma_start(bq2d[DEPTH:P, :], BQ[P:GW, :])
    bk01 = const.tile([P, 1], F32)
    nc.sync.dma_start(bk01[:], BK[0:P, :])
    bk2d = const.tile([P, 1], F32)
    nc.sync.dma_start(bk2d[0:DEPTH, :], BK[P:GW, :])
    nc.sync.dma_start(bk2d[DEPTH:P, :], BK[P:GW, :])
    # bv broadcast across partitions for the v-natural layout
    bvrow = const.tile([1, GW], F32)
    nc.sync.dma_start(bvrow[:], BV[:, :])
    bvb = const.tile([P, GW], F32)
    nc.gpsimd.partition_broadcast(bvb[:], bvrow[:])

    # weights, straight DMA into f32r tiles (same bits as f32)
    wq01 = const.tile([P, FCH, P], F32R)
    wq2d = const.tile([P, FCH, P], F32R)
    wk01 = const.tile([P, FCH, P], F32R)
    wk2d = const.tile([P, FCH, P], F32R)
    wre_q = WQ.rearrange("(c p) n -> p c n", p=P)
    wre_k = WK.rearrange("(c p) n -> p c n", p=P)
    nc.sync.dma_start(wq01[:], wre_q[:, :, 0:P])
    nc.sync.dma_start(wq2d[:, :, 0:DEPTH], wre_q[:, :, P:GW])
    nc.sync.dma_start(wq2d[:, :, DEPTH:P], wre_q[:, :, P:GW])
    nc.sync.dma_start(wk01[:], wre_k[:, :, 0:P])
    nc.sync.dma_start(wk2d[:, :, 0:DEPTH], wre_k[:, :, P:GW])
    nc.sync.dma_start(wk2d[:, :, DEPTH:P], wre_k[:, :, P:GW])
    wv = const.tile([P, FCH, 2 * GW], F32R)  # duplicated so N>=256
    wre_v = WV.rearrange("(c p) n -> p c n", p=P)
    nc.sync.dma_start(wv[:, :, 0:GW], wre_v)
    nc.sync.dma_start(wv[:, :, GW:2 * GW], wre_v)
    wo0 = const.tile([P, D], F32R)
    nc.sync.dma_start(wo0[:], WO[0:P, :])
    wo1 = const.tile([DEPTH, D], F32R)
    nc.sync.dma_start(wo1[:], WO[P:GW, :])

    # persistent attention operands
    qT01 = const.tile([P, S], F32R)
    qT2f = const.tile([P, S], F32R)   # head 2, duplicated partition halves
    kT01 = const.tile([P, S], F32R)
    kT2f = const.tile([P, S], F32R)
    vht = const.tile([P, NKC, HPC, DEPTH + 1], F32R)
    ones_t = const.tile([P, NKC], F32)
    nc.gpsimd.memset(ones_t[:], 1.0)
    for h in range(HPC):
        nc.vector.tensor_copy(vht[:, :, h, DEPTH], ones_t[:])
    hout01 = const.tile([P, S], F32R)
    hout2 = const.tile([DEPTH, S], F32R)

    return dict(
        ident=ident, bq01=bq01, bq2d=bq2d, bk01=bk01, bk2d=bk2d, bvb=bvb,
        wq01=wq01, wq2d=wq2d, wk01=wk01, wk2d=wk2d, wv=wv, wo0=wo0, wo1=wo1,
        qT01=qT01, qT2f=qT2f, kT01=kT01, kT2f=kT2f, vht=vht,
        hout01=hout01, hout2=hout2,
    )


def _emit_compute(nc, tc, tensors, st, phases="ABC"):
    _emit_phase_vk(nc, tc, tensors, st)
    _emit_interleaved(nc, tc, tensors, st)


def _emit_phase_vk(nc, tc, tensors, st):
    """Project V then K over the full sequence."""
    XQ, XK, XV, WQ, WK, WV, WO, BQ, BK, BV, OUT = tensors
    ident, bvb = st["ident"], st["bvb"]
    bk01, bk2d = st["bk01"], st["bk2d"]
    wk01, wk2d, wv = st["wk01"], st["wk2d"], st["wv"]
    kT01, kT2f, vht = st["kT01"], st["kT2f"], st["vht"]

    with (
        tc.tile_pool(name="xnat", bufs=3) as xnat_pool,
        tc.tile_pool(name="xts", bufs=2) as xts_pool,
        tc.tile_pool(name="tps", bufs=2, space="PSUM") as tps_pool,
        tc.tile_pool(name="pps", bufs=4, space="PSUM") as pps_pool,
    ):
        def transpose_sp(xre, sp):
            xn = xnat_pool.tile([P, 2, D], F32R, tag="xn", name="xn")
            nc.sync.dma_start(xn[:], xre[sp, :, :, :])
            xt = xts_pool.tile([P, FCH, 2 * P], F32R, tag="xt", name="xt")
            for a in range(2):
                tp = tps_pool.tile([P, FCH * P], F32R, tag="tp", name="tp")
                for f in range(FCH):
                    nc.tensor.transpose(
                        tp[:, f * P:(f + 1) * P],
                        xn[:, a, f * P:(f + 1) * P],
                        ident[:],
                    )
                nc.vector.tensor_copy(xt[:, :, a * P:(a + 1) * P], tp[:])
            return xt

        def proj_v(sp, xt):
            for a in range(2):
                pv = pps_pool.tile([P, 2 * GW], F32, tag="pp", name="pp")
                for f in range(FCH):
                    nc.tensor.matmul(
                        pv[:], xt[:, f, a * P:(a + 1) * P], wv[:, f, :],
                        start=(f == 0), stop=(f == FCH - 1),
                    )
                s = sp * 2 + a
                nc.vector.tensor_add(
                    vht[:, s, :, 0:DEPTH],
                    pv[:, 0:GW].rearrange("p (h d) -> p h d", h=HPC),
                    bvb[:].rearrange("p (h d) -> p h d", h=HPC),
                )

        def proj_k(sp, xt):
            sl = slice(sp * 2 * P, (sp + 1) * 2 * P)
            p01 = pps_pool.tile([P, 2 * P], F32, tag="pp", name="pp")
            for f in range(FCH):
                nc.tensor.matmul(
                    p01[:], wk01[:, f, :], xt[:, f, :],
                    start=(f == 0), stop=(f == FCH - 1),
                )
            p2d = pps_pool.tile([P, 2 * P], F32, tag="pp", name="pp")
            for f in range(FCH):
                nc.tensor.matmul(
                    p2d[:], wk2d[:, f, :], xt[:, f, :],
                    start=(f == 0), stop=(f == FCH - 1),
                )
            nc.vector.tensor_scalar_add(kT01[:, sl], p01[:], bk01[:])
            nc.vector.tensor_scalar_add(kT2f[:, sl], p2d[:], bk2d[:])

        xre_v = XV.rearrange("(sp a p) d -> sp p a d", a=2, p=P)
        xre_k = XK.rearrange("(sp a p) d -> sp p a d", a=2, p=P)
        steps = [("v", sp) for sp in range(NSP)] + \
                [("k", sp) for sp in range(NSP)]
        prev = None
        for kind, sp in steps:
            xt = transpose_sp(xre_v if kind == "v" else xre_k, sp)
            if prev is not None:
                (proj_v if prev[0] == "v" else proj_k)(prev[1], prev[2])
            prev = (kind, sp, xt)
        (proj_v if prev[0] == "v" else proj_k)(prev[1], prev[2])


def _emit_interleaved(nc, tc, tensors, st):
    """Per q-block: Q projection, attention (3 heads), prior block's
    output projection."""
    XQ, XK, XV, WQ, WK, WV, WO, BQ, BK, BV, OUT = tensors
    ident = st["ident"]
    bq01, bq2d = st["bq01"], st["bq2d"]
    wq01, wq2d, wo0, wo1 = st["wq01"], st["wq2d"], st["wo0"], st["wo1"]
    qT01, qT2f, kT01, kT2f = st["qT01"], st["qT2f"], st["kT01"], st["kT2f"]
    vht, hout01, hout2 = st["vht"], st["hout01"], st["hout2"]

    xre_q = XQ.rearrange("(sp a p) d -> sp p a d", a=2, p=P)

    with (
        tc.tile_pool(name="lg", bufs=2, space="PSUM") as lg_pool,
        tc.tile_pool(name="op", bufs=2, space="PSUM") as op_pool,
        tc.tile_pool(name="aux", bufs=2, space="PSUM") as aux_pool,
        tc.tile_pool(name="ex", bufs=3) as ex_pool,
        tc.tile_pool(name="nrm", bufs=4) as nrm_pool,
        tc.tile_pool(name="outt", bufs=3) as out_pool,
        tc.tile_pool(name="xnatq", bufs=2) as xnatq_pool,
        tc.tile_pool(name="xtsq", bufs=2) as xtsq_pool,
    ):
        def proj_q(sp):
            xn = xnatq_pool.tile([P, 2, D], F32R, tag="xn", name="xn")
            nc.sync.dma_start(xn[:], xre_q[sp, :, :, :])
            xt = xtsq_pool.tile([P, FCH, 2 * P], F32R, tag="xt", name="xt")
            for a in range(2):
                t0 = aux_pool.tile([P, 4 * P], F32R, tag="aux", name="t0")
                for f in range(4):
                    nc.tensor.transpose(
                        t0[:, f * P:(f + 1) * P],
                        xn[:, a, f * P:(f + 1) * P], ident[:])
                nc.vector.tensor_copy(
                    xt[:, 0:4, a * P:(a + 1) * P], t0[:])
                t1 = aux_pool.tile([P, 2 * P], F32R, tag="aux", name="t1")
                for f in range(2):
                    nc.tensor.transpose(
                        t1[:, f * P:(f + 1) * P],
                        xn[:, a, (4 + f) * P:(5 + f) * P], ident[:])
                nc.vector.tensor_copy(
                    xt[:, 4:6, a * P:(a + 1) * P], t1[:])
            sl = slice(sp * 2 * P, (sp + 1) * 2 * P)
            p01 = aux_pool.tile([P, 2 * P], F32, tag="aux", name="p01")
            for f in range(FCH):
                nc.tensor.matmul(
                    p01[:], wq01[:, f, :], xt[:, f, :],
                    start=(f == 0), stop=(f == FCH - 1),
                )
            nc.vector.tensor_scalar_add(qT01[:, sl], p01[:], bq01[:])
            p2d = aux_pool.tile([P, 2 * P], F32, tag="aux", name="p2d")
            for f in range(FCH):
                nc.tensor.matmul(
                    p2d[:], wq2d[:, f, :], xt[:, f, :],
                    start=(f == 0), stop=(f == FCH - 1),
                )
            nc.vector.tensor_scalar_add(qT2f[:, sl], p2d[:], bq2d[:])

        def normalize(outp, dst):
            rc = nrm_pool.tile([1, QB], F32, tag="rc", name="rc")
            nc.vector.reciprocal(rc[:], outp[DEPTH:DEPTH + 1, :])
            bc = nrm_pool.tile([DEPTH, QB], F32, tag="bc", name="bc")
            nc.gpsimd.partition_broadcast(bc[:], rc[:])
            nc.vector.tensor_mul(dst, outp[0:DEPTH, :], bc[:])

        def pass_h01(qb):
            qsl = slice(qb * QB, (qb + 1) * QB)
            outp0 = op_pool.tile([DEPTH + 1, QB], F32, tag="outp",
                                 name="outp0")
            outp1 = op_pool.tile([DEPTH + 1, QB], F32, tag="outp",
                                 name="outp1")

            def qk(kc):
                lg = lg_pool.tile([P, 2, QB], F32, tag="lg", name="lg")
                nc.tensor.matmul(
                    lg[:, 0, :], kT01[0:DEPTH, kc * P:(kc + 1) * P],
                    qT01[0:DEPTH, qsl], start=True, stop=True)
                nc.tensor.matmul(
                    lg[:, 1, :], kT01[DEPTH:P, kc * P:(kc + 1) * P],
                    qT01[DEPTH:P, qsl], start=True, stop=True)
                return lg

            def av(kc, lg):
                ext = ex_pool.tile([P, 2, QB], F32R, tag="ex", name="ex")
                nc.scalar.activation(ext[:], lg[:], AF.Exp, scale=SCALE)
                nc.tensor.matmul(
                    outp0[:], vht[:, kc, 0, :], ext[:, 0, :],
                    start=(kc == 0), stop=(kc == NKC - 1))
                nc.tensor.matmul(
                    outp1[:], vht[:, kc, 1, :], ext[:, 1, :],
                    start=(kc == 0), stop=(kc == NKC - 1))

            prev = qk(0)
            for kc in range(1, NKC):
                cur = qk(kc)
                av(kc - 1, prev)
                prev = cur
            av(NKC - 1, prev)
            normalize(outp0, hout01[0:DEPTH, qsl])
            normalize(outp1, hout01[DEPTH:P, qsl])

        def pass_h2(qb):
            qsl = slice(qb * QB, (qb + 1) * QB)
            outp2 = op_pool.tile([DEPTH + 1, QB], F32, tag="outp",
                                 name="outp2")

            def qk(j):
                lg = lg_pool.tile([P, 2, QB], F32, tag="lg", name="lg")
                nc.tensor.matmul(
                    lg[:, 0, :], kT2f[0:DEPTH, (2 * j) * P:(2 * j + 1) * P],
                    qT2f[0:DEPTH, qsl], start=True, stop=True)
                nc.tensor.matmul(
                    lg[:, 1, :],
                    kT2f[DEPTH:P, (2 * j + 1) * P:(2 * j + 2) * P],
                    qT2f[DEPTH:P, qsl], start=True, stop=True)
                return lg

            def av(j, lg):
                ext = ex_pool.tile([P, 2, QB], F32R, tag="ex", name="ex")
                nc.scalar.activation(ext[:], lg[:], AF.Exp, scale=SCALE)
                nc.tensor.matmul(
                    outp2[:], vht[:, 2 * j, 2, :], ext[:, 0, :],
                    start=(j == 0), stop=False)
                nc.tensor.matmul(
                    outp2[:], vht[:, 2 * j + 1, 2, :], ext[:, 1, :],
                    start=False, stop=(j == NKC // 2 - 1))

            prev = qk(0)
            for j in range(1, NKC // 2):
                cur = qk(j)
                av(j - 1, prev)
                prev = cur
            av(NKC // 2 - 1, prev)
            normalize(outp2, hout2[:, qsl])

        def emit_c(qb):
            for m in range(4 * qb, 4 * qb + 4):
                msl = slice(m * P, (m + 1) * P)
                pa = aux_pool.tile([P, 512], F32, tag="aux", name="pa")
                nc.tensor.matmul(pa[:], hout01[:, msl], wo0[:, 0:512],
                                 start=True, stop=False)
                nc.tensor.matmul(pa[:], hout2[:, msl], wo1[:, 0:512],
                                 start=False, stop=True)
                pb = aux_pool.tile([P, 256], F32, tag="aux", name="pb")
                nc.tensor.matmul(pb[:], hout01[:, msl], wo0[:, 512:D],
                                 start=True, stop=False)
                nc.tensor.matmul(pb[:], hout2[:, msl], wo1[:, 512:D],
                                 start=False, stop=True)
                ot = out_pool.tile([P, D], F32, tag="ot", name="ot")
                nc.vector.tensor_copy(ot[:, 0:512], pa[:])
                nc.vector.tensor_copy(ot[:, 512:D], pb[:])
                nc.sync.dma_start(OUT[msl, :], ot[:])

        for qb in range(NQB):
            proj_q(2 * qb)
            proj_q(2 * qb + 1)
            pass_h01(qb)
            if qb > 0:
                emit_c(qb - 1)
            pass_h2(qb)
        emit_c(NQB - 1)


_NC = None


def build_nc(repeat=1, phases="ABC"):
    nc = bacc.Bacc("TRN2", target_bir_lowering=False, debug=False)
    XQ = nc.dram_tensor("xq", [S, D], F32R, kind="ExternalInput").ap()
    XK = nc.dram_tensor("xk", [S, D], F32R, kind="ExternalInput").ap()
    XV = nc.dram_tensor("xv", [S, D], F32R, kind="ExternalInput").ap()
    WQ = nc.dram_tensor("wq", [D, GW], F32R, kind="ExternalInput").ap()
    WK = nc.dram_tensor("wk", [D, GW], F32R, kind="ExternalInput").ap()
    WV = nc.dram_tensor("wv", [D, GW], F32R, kind="ExternalInput").ap()
    WO = nc.dram_tensor("wo", [GW, D], F32R, kind="ExternalInput").ap()
    BQ = nc.dram_tensor("bq", [GW, 1], F32, kind="ExternalInput").ap()
    BK = nc.dram_tensor("bk", [GW, 1], F32, kind="ExternalInput").ap()
    BV = nc.dram_tensor("bv", [1, GW], F32, kind="ExternalInput").ap()
    OUT = nc.dram_tensor("out", [S, D], F32, kind="ExternalOutput").ap()
    tensors = (XQ, XK, XV, WQ, WK, WV, WO, BQ, BK, BV, OUT)
    from contextlib import ExitStack
    with tile.TileContext(nc) as tc:
        with ExitStack() as ctx:
            _emit(nc, tc, ctx, tensors, repeat=repeat, phases=phases)
    nc.compile()
    return nc


def _get_nc():
    global _NC
    if _NC is None:
        _NC = build_nc()
    return _NC


def kernel(**inputs):
    global LAST_RESULTS
    q = np.ascontiguousarray(np.asarray(inputs["q"], dtype=np.float32))
    k = np.ascontiguousarray(np.asarray(inputs["k"], dtype=np.float32))
    v = np.ascontiguousarray(np.asarray(inputs["v"], dtype=np.float32))
    Wq = np.asarray(inputs["Wq"], dtype=np.float32)
    Wk = np.asarray(inputs["Wk"], dtype=np.float32)
    Wv = np.asarray(inputs["Wv"], dtype=np.float32)
    Wo = np.asarray(inputs["Wo"], dtype=np.float32)
    bq = np.asarray(inputs["bq"], dtype=np.float32)
    bk = np.asarray(inputs["bk"], dtype=np.float32)
    bv = np.asarray(inputs["bv"], dtype=np.float32)
    bo = np.asarray(inputs["bo"], dtype=np.float32)
    # mask is all zeros by problem spec; ignored.

    nc = _get_nc()
    in_maps = []
    for c in range(N_CORES):
        b, g = c // 4, c % 4
        sl = slice(g * GW, (g + 1) * GW)
        in_maps.append({
            "xq": q[b], "xk": k[b], "xv": v[b],
            "wq": np.ascontiguousarray(Wq[:, sl]),
            "wk": np.ascontiguousarray(Wk[:, sl]),
            "wv": np.ascontiguousarray(Wv[:, sl]),
            "wo": np.ascontiguousarray(Wo[sl, :]),
            "bq": np.ascontiguousarray(bq[sl].reshape(GW, 1)),
            "bk": np.ascontiguousarray(bk[sl].reshape(GW, 1)),
            "bv": np.ascontiguousarray(bv[sl].reshape(1, GW)),
        })
    kwargs = {}
    if TRACE:
        kwargs = dict(trace=True)
    res = bass_utils.run_bass_kernel_spmd(nc, in_maps, list(range(N_CORES)),
                                          **kwargs)
    LAST_RESULTS = res
    out = np.zeros((B, S, D), dtype=np.float32)
    for c in range(N_CORES):
        out[c // 4] += res.results[c]["out"]
    out += bo
    return out


# revision 6
# speedup vs baseline: 610.4090x; 610.4090x over previous
"""GPT-3 style multi-head attention on Trainium2, 8-core SPMD Bass kernel.

Problem shapes: B=2, S=4096, D=768, H=12, depth=64 (fp32).

Sharding (hardcoded): core c in 0..7 -> batch b = c//4, head group g = c%4
(3 heads per core).  Per core, v2 schedule (ACT/exp is the bottleneck
engine, everything else hides under it):
  1. project V then K for all seq (PE transpose in f32r + f32r matmuls,
     DVE evacuations with fused bias),
  2. per 512-wide q-block: project Q for that block, then attention:
     QK^T for heads 0&1 issued as a row-tiled pair (K=64 each, PE array
     halves run concurrently), exp on ScalarE over both heads in one
     instruction, AV with an appended ones column for the softmax
     denominator; head 2 is self-paired over (even, odd) key chunks
     using partition-duplicated qT/kT built for free via duplicated
     weight columns,
  3. output projection emitted per q-block, interleaved under the next
     block's exp shadow; partial [4096, 768] -> DRAM.
Host sums the 4 partials per batch and adds the output bias bo.
"""

import numpy as np

import concourse.bacc as bacc
import concourse.mybir as mybir
import concourse.tile as tile
from concourse import bass_utils
from concourse.masks import make_identity

B, S, D, H = 2, 4096, 768, 12
DEPTH = 64
HPC = 3                 # heads per core
GW = HPC * DEPTH        # 192: head-group width
N_CORES = 8
SCALE = 1.0 / float(np.sqrt(DEPTH))

F32 = mybir.dt.float32
F32R = mybir.dt.float32r
AF = mybir.ActivationFunctionType

P = 128
FCH = D // P            # 6 feature chunks
NSP = S // (2 * P)      # 16 seq pairs (256 rows each)
NKC = S // P            # 32 key chunks
QB = 512                # q block width
NQB = S // QB           # 8

# set by test.py to get a traced run
TRACE = False
LAST_RESULTS = None


def _emit(nc, tc, ctx, tensors, repeat=1, phases="ABC"):
    setup = _emit_setup(nc, tc, ctx, tensors)
    for _ in range(repeat):
        _emit_compute(nc, tc, tensors, setup, phases=phases)


def _emit_setup(nc, tc, ctx, tensors):
    XQ, XK, XV, WQ, WK, WV, WO, BQ, BK, BV, OUT = tensors

    const = ctx.enter_context(tc.tile_pool(name="const", bufs=1))

    ident_f = const.tile([P, P], F32)
    make_identity(nc, ident_f[:])
    ident_r = const.tile([P, P], F32R)
    nc.vector.tensor_copy(ident_r[:], ident_f[:])
    ident = ident_r[:]

    # biases as per-partition columns; head-2 slices duplicated into both
    # partition halves
    bq01 = const.tile([P, 1], F32)
    nc.sync.dma_start(bq01[:], BQ[0:P, :])
    bq2d = const.tile([P, 1], F32)
    nc.sync.dma_start(bq2d[0:DEPTH, :], BQ[P:GW, :])
    nc.sync.dma_start(bq2d[DEPTH:P, :], BQ[P:GW, :])
    bk01 = const.tile([P, 1], F32)
    nc.sync.dma_start(bk01[:], BK[0:P, :])
    bk2d = const.tile([P, 1], F32)
    nc.sync.dma_start(bk2d[0:DEPTH, :], BK[P:GW, :])
    nc.sync.dma_start(bk2d[DEPTH:P, :], BK[P:GW, :])
    # bv broadcast across partitions for the v-natural layout
    bvrow = const.tile([1, GW], F32)
    nc.sync.dma_start(bvrow[:], BV[:, :])
    bvb = const.tile([P, GW], F32)
    nc.gpsimd.partition_broadcast(bvb[:], bvrow[:])

    # weights, straight DMA into f32r tiles (same bits as f32)
    wq01 = const.tile([P, FCH, P], F32R)
    wq2d = const.tile([P, FCH, P], F32R)
    wk01 = const.tile([P, FCH, P], F32R)
    wk2d = const.tile([P, FCH, P], F32R)
    wre_q = WQ.rearrange("(c p) n -> p c n", p=P)
    wre_k = WK.rearrange("(c p) n -> p c n", p=P)
    nc.sync.dma_start(wq01[:], wre_q[:, :, 0:P])
    nc.sync.dma_start(wq2d[:, :, 0:DEPTH], wre_q[:, :, P:GW])
    nc.sync.dma_start(wq2d[:, :, DEPTH:P], wre_q[:, :, P:GW])
    nc.sync.dma_start(wk01[:], wre_k[:, :, 0:P])
    nc.sync.dma_start(wk2d[:, :, 0:DEPTH], wre_k[:, :, P:GW])
    nc.sync.dma_start(wk2d[:, :, DEPTH:P], wre_k[:, :, P:GW])
    wv = const.tile([P, FCH, 2 * GW], F32R)  # duplicated so N>=256
    wre_v = WV.rearrange("(c p) n -> p c n", p=P)
    nc.sync.dma_start(wv[:, :, 0:GW], wre_v)
    nc.sync.dma_start(wv[:, :, GW:2 * GW], wre_v)
    wo0 = const.tile([P, D], F32R)
    nc.sync.dma_start(wo0[:], WO[0:P, :])
    wo1 = const.tile([DEPTH, D], F32R)
    nc.sync.dma_start(wo1[:], WO[P:GW, :])

    # persistent attention operands
    qT01 = const.tile([P, S], F32R)
    qT2f = const.tile([P, S], F32R)   # head 2, duplicated partition halves
    kT01 = const.tile([P, S], F32R)
    kT2f = const.tile([P, S], F32R)
    vht = const.tile([P, NKC, HPC, DEPTH + 1], F32R)
    ones_t = const.tile([P, NKC], F32)
    nc.gpsimd.memset(ones_t[:], 1.0)
    for h in range(HPC):
        nc.vector.tensor_copy(vht[:, :, h, DEPTH], ones_t[:])
    hout01 = const.tile([P, S], F32R)
    hout2 = const.tile([DEPTH, S], F32R)

    return dict(
        ident=ident, bq01=bq01, bq2d=bq2d, bk01=bk01, bk2d=bk2d, bvb=bvb,
        wq01=wq01, wq2d=wq2d, wk01=wk01, wk2d=wk2d, wv=wv, wo0=wo0, wo1=wo1,
        qT01=qT01, qT2f=qT2f, kT01=kT01, kT2f=kT2f, vht=vht,
        hout01=hout01, hout2=hout2,
    )


def _emit_compute(nc, tc, tensors, st, phases="ABC"):
    _emit_phase_vk(nc, tc, tensors, st)
    _emit_interleaved(nc, tc, tensors, st)


def _emit_phase_vk(nc, tc, tensors, st):
    """Project V then K over the full sequence."""
    XQ, XK, XV, WQ, WK, WV, WO, BQ, BK, BV, OUT = tensors
    ident, bvb = st["ident"], st["bvb"]
    bk01, bk2d = st["bk01"], st["bk2d"]
    wk01, wk2d, wv = st["wk01"], st["wk2d"], st["wv"]
    kT01, kT2f, vht = st["kT01"], st["kT2f"], st["vht"]

    with (
        tc.tile_pool(name="xnat", bufs=3) as xnat_pool,
        tc.tile_pool(name="xts", bufs=2) as xts_pool,
        tc.tile_pool(name="tps", bufs=2, space="PSUM") as tps_pool,
        tc.tile_pool(name="pps", bufs=4, space="PSUM") as pps_pool,
    ):
        def transpose_sp(xre, sp):
            xn = xnat_pool.tile([P, 2, D], F32R, tag="xn", name="xn")
            nc.sync.dma_start(xn[:], xre[sp, :, :, :])
            xt = xts_pool.tile([P, FCH, 2 * P], F32R, tag="xt", name="xt")
            for a in range(2):
                tp = tps_pool.tile([P, FCH * P], F32R, tag="tp", name="tp")
                for f in range(FCH):
                    nc.tensor.transpose(
                        tp[:, f * P:(f + 1) * P],
                        xn[:, a, f * P:(f + 1) * P],
                        ident,
                    )
                nc.vector.tensor_copy(xt[:, :, a * P:(a + 1) * P], tp[:])
            return xt

        def proj_v(sp, xt):
            for a in range(2):
                pv = pps_pool.tile([P, 2 * GW], F32, tag="pp", name="pp")
                for f in range(FCH):
                    nc.tensor.matmul(
                        pv[:], xt[:, f, a * P:(a + 1) * P], wv[:, f, :],
                        start=(f == 0), stop=(f == FCH - 1),
                    )
                s = sp * 2 + a
                nc.vector.tensor_add(
                    vht[:, s, :, 0:DEPTH],
                    pv[:, 0:GW].rearrange("p (h d) -> p h d", h=HPC),
                    bvb[:].rearrange("p (h d) -> p h d", h=HPC),
                )

        def proj_k(sp, xt):
            sl = slice(sp * 2 * P, (sp + 1) * 2 * P)
            p01 = pps_pool.tile([P, 2 * P], F32, tag="pp", name="pp")
            for f in range(FCH):
                nc.tensor.matmul(
                    p01[:], wk01[:, f, :], xt[:, f, :],
                    start=(f == 0), stop=(f == FCH - 1),
                )
            p2d = pps_pool.tile([P, 2 * P], F32, tag="pp", name="pp")
            for f in range(FCH):
                nc.tensor.matmul(
                    p2d[:], wk2d[:, f, :], xt[:, f, :],
                    start=(f == 0), stop=(f == FCH - 1),
                )
            nc.vector.tensor_scalar_add(kT01[:, sl], p01[:], bk01[:])
            nc.vector.tensor_scalar_add(kT2f[:, sl], p2d[:], bk2d[:])

        xre_v = XV.rearrange("(sp a p) d -> sp p a d", a=2, p=P)
        xre_k = XK.rearrange("(sp a p) d -> sp p a d", a=2, p=P)
        steps = [("v", sp) for sp in range(NSP)] + \
                [("k", sp) for sp in range(NSP)]
        prev = None
        for kind, sp in steps:
            xt = transpose_sp(xre_v if kind == "v" else xre_k, sp)
            if prev is not None:
                (proj_v if prev[0] == "v" else proj_k)(prev[1], prev[2])
            prev = (kind, sp, xt)
        (proj_v if prev[0] == "v" else proj_k)(prev[1], prev[2])


def _emit_interleaved(nc, tc, tensors, st):
    """Per q-block: Q projection, attention (3 heads), prior block's
    output projection."""
    XQ, XK, XV, WQ, WK, WV, WO, BQ, BK, BV, OUT = tensors
    ident = st["ident"]
    bq01, bq2d = st["bq01"], st["bq2d"]
    wq01, wq2d, wo0, wo1 = st["wq01"], st["wq2d"], st["wo0"], st["wo1"]
    qT01, qT2f, kT01, kT2f = st["qT01"], st["qT2f"], st["kT01"], st["kT2f"]
    vht, hout01, hout2 = st["vht"], st["hout01"], st["hout2"]

    xre_q = XQ.rearrange("(sp a p) d -> sp p a d", a=2, p=P)

    with (
        tc.tile_pool(name="lg", bufs=2, space="PSUM") as lg_pool,
        tc.tile_pool(name="op", bufs=2, space="PSUM") as op_pool,
        tc.tile_pool(name="aux", bufs=2, space="PSUM") as aux_pool,
        tc.tile_pool(name="ex", bufs=3) as ex_pool,
        tc.tile_pool(name="nrm", bufs=2) as nrm_pool,
        tc.tile_pool(name="outt", bufs=2) as out_pool,
        tc.tile_pool(name="xnatq", bufs=2) as xnatq_pool,
        tc.tile_pool(name="xtsq", bufs=2) as xtsq_pool,
    ):
        def proj_q(sp):
            xn = xnatq_pool.tile([P, 2, D], F32R, tag="xn", name="xn")
            nc.sync.dma_start(xn[:], xre_q[sp, :, :, :])
            xt = xtsq_pool.tile([P, FCH, 2 * P], F32R, tag="xt", name="xt")
            for a in range(2):
                t0 = aux_pool.tile([P, 4 * P], F32R, tag="aux", name="t0")
                for f in range(4):
                    nc.tensor.transpose(
                        t0[:, f * P:(f + 1) * P],
                        xn[:, a, f * P:(f + 1) * P], ident)
                nc.vector.tensor_copy(
                    xt[:, 0:4, a * P:(a + 1) * P], t0[:])
                t1 = aux_pool.tile([P, 2 * P], F32R, tag="aux", name="t1")
                for f in range(2):
                    nc.tensor.transpose(
                        t1[:, f * P:(f + 1) * P],
                        xn[:, a, (4 + f) * P:(5 + f) * P], ident)
                nc.vector.tensor_copy(
                    xt[:, 4:6, a * P:(a + 1) * P], t1[:])
            sl = slice(sp * 2 * P, (sp + 1) * 2 * P)
            p01 = aux_pool.tile([P, 2 * P], F32, tag="aux", name="p01")
            for f in range(FCH):
                nc.tensor.matmul(
                    p01[:], wq01[:, f, :], xt[:, f, :],
                    start=(f == 0), stop=(f == FCH - 1),
                )
            nc.vector.tensor_scalar_add(qT01[:, sl], p01[:], bq01[:])
            p2d = aux_pool.tile([P, 2 * P], F32, tag="aux", name="p2d")
            for f in range(FCH):
                nc.tensor.matmul(
                    p2d[:], wq2d[:, f, :], xt[:, f, :],
                    start=(f == 0), stop=(f == FCH - 1),
                )
            nc.vector.tensor_scalar_add(qT2f[:, sl], p2d[:], bq2d[:])

        def normalize(outp, dst):
            rc = nrm_pool.tile([1, QB], F32, tag="rc", name="rc")
            nc.vector.reciprocal(rc[:], outp[DEPTH:DEPTH + 1, :])
            bc = nrm_pool.tile([DEPTH, QB], F32, tag="bc", name="bc")
            nc.gpsimd.partition_broadcast(bc[:], rc[:])
            nc.vector.tensor_mul(dst, outp[0:DEPTH, :], bc[:])

        def pass_h01(qb):
            qsl = slice(qb * QB, (qb + 1) * QB)
            outp0 = op_pool.tile([DEPTH + 1, QB], F32, tag="outp",
                                 name="outp0")
            outp1 = op_pool.tile([DEPTH + 1, QB], F32, tag="outp",
                                 name="outp1")

            def qk(kc):
                lg = lg_pool.tile([P, 2, QB], F32, tag="lg", name="lg")
                nc.tensor.matmul(
                    lg[:, 0, :], kT01[0:DEPTH, kc * P:(kc + 1) * P],
                    qT01[0:DEPTH, qsl], start=True, stop=True)
                nc.tensor.matmul(
                    lg[:, 1, :], kT01[DEPTH:P, kc * P:(kc + 1) * P],
                    qT01[DEPTH:P, qsl], start=True, stop=True)
                return lg

            def av(kc, lg):
                ext = ex_pool.tile([P, 2, QB], F32R, tag="ex", name="ex")
                nc.scalar.activation(ext[:], lg[:], AF.Exp, scale=SCALE)
                nc.tensor.matmul(
                    outp0[:], vht[:, kc, 0, :], ext[:, 0, :],
                    start=(kc == 0), stop=(kc == NKC - 1))
                nc.tensor.matmul(
                    outp1[:], vht[:, kc, 1, :], ext[:, 1, :],
                    start=(kc == 0), stop=(kc == NKC - 1))

            prev = qk(0)
            for kc in range(1, NKC):
                cur = qk(kc)
                av(kc - 1, prev)
                prev = cur
            av(NKC - 1, prev)
            normalize(outp0, hout01[0:DEPTH, qsl])
            normalize(outp1, hout01[DEPTH:P, qsl])

        def pass_h2(qb):
            qsl = slice(qb * QB, (qb + 1) * QB)
            outp2 = op_pool.tile([DEPTH + 1, QB], F32, tag="outp",
                                 name="outp2")

            def qk(j):
                lg = lg_pool.tile([P, 2, QB], F32, tag="lg", name="lg")
                nc.tensor.matmul(
                    lg[:, 0, :], kT2f[0:DEPTH, (2 * j) * P:(2 * j + 1) * P],
                    qT2f[0:DEPTH, qsl], start=True, stop=True)
                nc.tensor.matmul(
                    lg[:, 1, :],
                    kT2f[DEPTH:P, (2 * j + 1) * P:(2 * j + 2) * P],
                    qT2f[DEPTH:P, qsl], start=True, stop=True)
                return lg

            def av(j, lg):
                ext = ex_pool.tile([P, 2, QB], F32R, tag="ex", name="ex")
                nc.scalar.activation(ext[:], lg[:], AF.Exp, scale=SCALE)
                nc.tensor.matmul(
                    outp2[:], vht[:, 2 * j, 2, :], ext[:, 0, :],
                    start=(j == 0), stop=False)
                nc.tensor.matmul(
                    outp2[:], vht[:, 2 * j + 1, 2, :], ext[:, 1, :],
                    start=False, stop=(j == NKC // 2 - 1))

            prev = qk(0)
            for j in range(1, NKC // 2):
                cur = qk(j)
                av(j - 1, prev)
                prev = cur
            av(NKC // 2 - 1, prev)
            normalize(outp2, hout2[:, qsl])

        def emit_c(qb):
            for m in range(4 * qb, 4 * qb + 4):
                msl = slice(m * P, (m + 1) * P)
                pa = aux_pool.tile([P, 512], F32, tag="aux", name="pa")
                nc.tensor.matmul(pa[:], hout01[:, msl], wo0[:, 0:512],
                                 start=True, stop=False)
                nc.tensor.matmul(pa[:], hout2[:, msl], wo1[:, 0:512],
                                 start=False, stop=True)
                pb = aux_pool.tile([P, 256], F32, tag="aux", name="pb")
                nc.tensor.matmul(pb[:], hout01[:, msl], wo0[:, 512:D],
                                 start=True, stop=False)
                nc.tensor.matmul(pb[:], hout2[:, msl], wo1[:, 512:D],
                                 start=False, stop=True)
                ot = out_pool.tile([P, D], F32, tag="ot", name="ot")
                nc.vector.tensor_copy(ot[:, 0:512], pa[:])
                nc.vector.tensor_copy(ot[:, 512:D], pb[:])
                nc.sync.dma_start(OUT[msl, :], ot[:])

        for qb in range(NQB):
            proj_q(2 * qb)
            proj_q(2 * qb + 1)
            pass_h01(qb)
            if qb > 0:
                emit_c(qb - 1)
            pass_h2(qb)
        emit_c(NQB - 1)


_NC = None


def build_nc(repeat=1, phases="ABC"):
    nc = bacc.Bacc("TRN2", target_bir_lowering=False, debug=False)
    XQ = nc.dram_tensor("xq", [S, D], F32R, kind="ExternalInput").ap()
    XK = nc.dram_tensor("xk", [S, D], F32R, kind="ExternalInput").ap()
    XV = nc.dram_tensor("xv", [S, D], F32R, kind="ExternalInput").ap()
    WQ = nc.dram_tensor("wq", [D, GW], F32R, kind="ExternalInput").ap()
    WK = nc.dram_tensor("wk", [D, GW], F32R, kind="ExternalInput").ap()
    WV = nc.dram_tensor("wv", [D, GW], F32R, kind="ExternalInput").ap()
    WO = nc.dram_tensor("wo", [GW, D], F32R, kind="ExternalInput").ap()
    BQ = nc.dram_tensor("bq", [GW, 1], F32, kind="ExternalInput").ap()
    BK = nc.dram_tensor("bk", [GW, 1], F32, kind="ExternalInput").ap()
    BV = nc.dram_tensor("bv", [1, GW], F32, kind="ExternalInput").ap()
    OUT = nc.dram_tensor("out", [S, D], F32, kind="ExternalOutput").ap()
    tensors = (XQ, XK, XV, WQ, WK, WV, WO, BQ, BK, BV, OUT)
    from contextlib import ExitStack
    with tile.TileContext(nc) as tc:
        with ExitStack() as ctx:
            _emit(nc, tc, ctx, tensors, repeat=repeat, phases=phases)
    nc.compile()
    return nc


def _get_nc():
    global _NC
    if _NC is None:
        _NC = build_nc()
    return _NC


def kernel(**inputs):
    global LAST_RESULTS
    q = np.ascontiguousarray(np.asarray(inputs["q"], dtype=np.float32))
    k = np.ascontiguousarray(np.asarray(inputs["k"], dtype=np.float32))
    v = np.ascontiguousarray(np.asarray(inputs["v"], dtype=np.float32))
    Wq = np.asarray(inputs["Wq"], dtype=np.float32)
    Wk = np.asarray(inputs["Wk"], dtype=np.float32)
    Wv = np.asarray(inputs["Wv"], dtype=np.float32)
    Wo = np.asarray(inputs["Wo"], dtype=np.float32)
    bq = np.asarray(inputs["bq"], dtype=np.float32)
    bk = np.asarray(inputs["bk"], dtype=np.float32)
    bv = np.asarray(inputs["bv"], dtype=np.float32)
    bo = np.asarray(inputs["bo"], dtype=np.float32)
    # mask is all zeros by problem spec; ignored.

    nc = _get_nc()
    in_maps = []
    for c in range(N_CORES):
        b, g = c // 4, c % 4
        sl = slice(g * GW, (g + 1) * GW)
        in_maps.append({
            "xq": q[b], "xk": k[b], "xv": v[b],
            "wq": np.ascontiguousarray(Wq[:, sl]),
            "wk": np.ascontiguousarray(Wk[:, sl]),
            "wv": np.ascontiguousarray(Wv[:, sl]),
            "wo": np.ascontiguousarray(Wo[sl, :]),
            "bq": np.ascontiguousarray(bq[sl].reshape(GW, 1)),
            "bk": np.ascontiguousarray(bk[sl].reshape(GW, 1)),
            "bv": np.ascontiguousarray(bv[sl].reshape(1, GW)),
        })
    kwargs = {}
    if TRACE:
        kwargs = dict(trace=True)
    res = bass_utils.run_bass_kernel_spmd(nc, in_maps, list(range(N_CORES)),
                                          **kwargs)
    LAST_RESULTS = res
    out = np.zeros((B, S, D), dtype=np.float32)
    for c in range(N_CORES):
        out[c // 4] += res.results[c]["out"]
    out += bo
    return out
